# revision 40
# baseline (speedup 1.0000x reference)
"""Trainium2 Bass kernel: per-element maximization of the lognormal-CDF
surplus  s(d) = bid*(1-d)*Phi((ln(d*bid)-mu)/sigma)  over d in (0,1).

Algorithm change vs the GSS reference: the surplus is strictly unimodal in d
(its log-derivative is a decreasing function), so the reference's 20-iteration
golden-section search output is within phi^-20 ~ 6.6e-5 of the root of the
first-order condition.  We binary-search the FOC sign directly:

    sign(ds/dd) = sign( (1-d)*B*E(t) - (1+erf(t))*d ),
    t = (ln d + A2)*B,  A2 = ln(bid)-mu,  B = 1/(sigma*sqrt2),
    E(t) = (2/sqrt(pi))*exp(-t^2)

One probe per iteration (vs two surplus evaluations for GSS) and 0.5x interval
shrink per iteration (vs 0.618x) means 7 probes reach 2^-8 ~ 3.9e-3 accuracy;
measured rel_l2 vs the reference is 5.6e-3 on the full 16.7M-element input
(tolerance 2e-2).  Ties (both FOC terms underflow to 0 when the CDF is fully
saturated) step right, matching the reference's cond=False branch.  Probe 0 is
at the constant d=0.5, so its Ln folds into a tensor_scalar and its decision
needs no mid factors.

Engine mapping per iteration per [128, FD] chunk:
  ACT: Ln(mid) -> bf16, E=Derivative_Erf(t) -> bf16, Erf(t) -> fp32
       (3 table sets; get_activation_tables is patched - contents only,
        order/ids preserved - so the chooser never bounces between the
        per-function home sets)
  VE : t=(L+a2)*b (2 bf16 tensor_tensor at 2x), u=b*E (bf16 2x),
       cmp1=(mid-1)*u -> bf16 (stt, fp32 internal), cmp2=(1+erf)*mid -> bf16
       (stt; the +1 must precede any bf16 rounding - quantizing erf directly
        cancels catastrophically near erf=-1), fused custom-DVE update
       mid' = (mid - w) + (g<=0)*2w reading g from PSUM (1 op)
  PE : g = cmp1 + cmp2 as identity-weight bf16 matmuls accumulated in PSUM
       (exact fp32 sum of the bf16 operands; TensorE is otherwise idle)
mid stays fp32 (needs 2^-9 resolution).  GpSimd is only used for memsets:
its elementwise ops share the DVE SBUF port and inflate Vector op latency.
Chunks are processed in interleaved pairs, stages interleaved across the
pair, so ACT/PE work on one chunk hides under VE work on the other.
"""
import sys

sys.path.insert(0, "/opt/trn_rl_repo")

import numpy as np

N_TOTAL = 16777216
N_CORES = 8
N_PER_CORE = N_TOTAL // N_CORES  # 2097152
P = 128
FD = 1024
GROUP = 4
N_ITER = 7

_CUSTOM_OP = None


def _get_custom_op():
    """Register the fused bisection-update DVE op (idempotent)."""
    global _CUSTOM_OP
    if _CUSTOM_OP is not None:
        return _CUSTOM_OP
    import concourse.dve_ops as dops
    from concourse.dve_spec import Spec, Src0, Src1, C1, C2, Zero, lower
    from concourse.dve_uop import DveOpSpec

    name = "BISECT_STEP_ANT"
    if name in dops._SUB_OPCODE_FOR_NAME:
        _CUSTOM_OP = next(op for op in dops.OPS if op.name == name)
        return _CUSTOM_OP

    body = (Src1 - C1) + (Src0 <= Zero) * C2

    def _ref(in0, in1, s0, s1, imm2):
        return ((in1.astype(np.float32) - s1)
                + (in0 <= 0.0).astype(np.float32) * imm2).astype(np.float32)

    spec = Spec(body=body, reference=_ref)
    row = max(dops._SUB_OPCODE_FOR_NAME.values()) + 1
    assert row < 0x20
    shas = {}
    for ver in ("v3", "v4"):
        uops = lower(spec, ver=ver)
        shas[ver] = DveOpSpec(
            name=name, opcode=row, uops=uops, rd1_en=True
        ).sha(ver)
    op = dops.DveOp(name, spec, subdim=False, uops_sha=shas)
    dops.OPS.append(op)
    dops.CUSTOM_DVE_SPECS[name] = spec
    dops._SUB_OPCODE_FOR_NAME[name] = row
    _CUSTOM_OP = op
    return op


def _patch_act_table_order():
    """Make the act-table chooser use one set for Ln+Square+Exp.

    The dict insertion order of get_activation_tables IS the
    act_func_set_id walrus uses, so the dict may not be reordered.
    Instead, prune Ln/Square/Exp from every set except
    natural_log_exp_and_others (and Square also stays in
    sigmoid_and_others, Erf's set) so the chooser has no other choice.
    The physical tables are unchanged — we only narrow the chooser's
    view — so the pruned functions still work from the kept sets."""
    import concourse.bacc as bacc
    import concourse.hw_specs as hw
    import concourse.mybir as mybir

    if getattr(bacc, "_ant_table_patched", False):
        return
    orig = hw.get_activation_tables
    AF = mybir.ActivationFunctionType

    def pruned(arch):
        t = {k: set(v) for k, v in orig(arch).items()}
        keep_ln_exp = "natural_log_exp_and_others"
        for name, fns in t.items():
            if name != keep_ln_exp:
                fns.discard(AF.Ln)
                fns.discard(AF.Exp)
                fns.discard(AF.Square)
        return t

    bacc.get_activation_tables = pruned
    bacc._ant_table_patched = True


def _build_nc(n_per_core, fd, group_size=GROUP, n_iter=N_ITER):
    import concourse.bass as bass
    import concourse.bacc as bacc
    import concourse.mybir as mybir
    import concourse.tile as tile

    _patch_act_table_order()

    from concourse import masks

    AF = mybir.ActivationFunctionType
    ALU = mybir.AluOpType
    dt = mybir.dt.float32
    dtb = mybir.dt.bfloat16

    step_op = _get_custom_op()

    n_chunks = n_per_core // (P * fd)
    assert n_chunks * P * fd == n_per_core

    nc = bacc.Bacc(None, target_bir_lowering=False)

    exp_bias = float(np.log(2.0 / np.sqrt(np.pi)))

    # non-Copy activation float biases must exist as const APs
    def register_const(value: float):
        if (dt, value) in nc.const_aps.aps:
            return
        t = nc.alloc_sbuf_tensor(f"const-f32-c{len(nc.const_aps.aps)}", [128, 1], dt)
        nc.gpsimd.memset(t.ap(), value)
        nc.const_aps.aps[(dt, value)] = t.ap()

    register_const(exp_bias)
    nc.all_engine_barrier()

    params = nc.declare_dram_parameter("params", [n_per_core, 2], dt, isOutput=False)
    bids = nc.declare_dram_parameter("bids", [n_per_core], dt, isOutput=False)
    out = nc.declare_dram_parameter("out", [n_per_core], dt, isOutput=True)

    # contiguous [G, 128, 2*fd] view of interleaved (mu, sigma) pairs
    params_v = params.rearrange("(g p f) c -> g p (f c)", p=P, f=fd)
    bids_v = bids.rearrange("(g p f) -> g p f", p=P, f=fd)
    out_v = out.rearrange("(g p f) -> g p f", p=P, f=fd)

    sqrt2 = float(np.sqrt(2.0))

    n_blk = fd // 512  # PSUM-bank-sized matmul blocks

    with tile.TileContext(nc) as tc:
        with (
            tc.tile_pool(name="ident", bufs=3) as p_id,
            tc.tile_pool(name="st_mid", bufs=2 * group_size) as p_mid,
            tc.tile_pool(name="st_a2", bufs=2 * group_size) as p_a2,
            tc.tile_pool(name="st_b", bufs=2 * group_size) as p_b,
            tc.tile_pool(name="t1", bufs=group_size + 1) as p_t1,
            tc.tile_pool(name="t2", bufs=group_size) as p_t2,
            tc.tile_pool(name="t3", bufs=group_size + 1) as p_t3,
            tc.tile_pool(name="c1", bufs=group_size + 1) as p_c1,
            tc.tile_pool(name="gp", bufs=group_size, space="PSUM") as p_g,
            tc.tile_pool(name="sa", bufs=group_size) as p_sa,
            tc.tile_pool(name="sb", bufs=group_size) as p_sb,
            tc.tile_pool(name="pload", bufs=group_size - 1) as p_pl,
        ):
            ident = p_id.tile([P, P], dtb, tag="ident")
            masks.make_identity(nc, ident[:])
            identf = p_id.tile([P, P], dt, tag="identf")
            masks.make_identity(nc, identf[:])
            # negated identity: g = I*P + (-I)*u in one PSUM accumulation
            identn = p_id.tile([P, P], dtb, tag="identn")
            nc.gpsimd.memset(identn[:], 0.0)
            nc.gpsimd.affine_select(
                out=identn[:], in_=identn[:],
                compare_op=ALU.not_equal, fill=-1.0,
                base=0, pattern=[[-1, P]], channel_multiplier=1,
            )
            for g0 in range(0, n_chunks, group_size):
                members = []
                # ---- setup each chunk of the group ----
                for gi in range(g0, min(g0 + group_size, n_chunks)):
                    mid = p_mid.tile([P, fd], dt, tag="mid")
                    a2 = p_a2.tile([P, fd], dtb, tag="a2")
                    bt = p_b.tile([P, fd], dtb, tag="b")
                    sa = p_sa.tile([P, fd], dt, tag="sa")
                    sb = p_sb.tile([P, fd], dt, tag="sb")
                    # load bid -> sa (scratch), then logbid in place
                    nc.sync.dma_start(sa[:], bids_v[gi])
                    nc.scalar.activation(sa[:], sa[:], AF.Ln)
                    # interleaved params arrive in two [P, fd] staging halves
                    for h in range(2):
                        pl = p_pl.tile([P, fd], dt, tag="pl")
                        nc.sync.dma_start(pl[:], params_v[gi, :, h * fd:(h + 1) * fd])
                        plv = pl.rearrange("p (f c) -> p f c", c=2)
                        half = slice(h * (fd // 2), (h + 1) * (fd // 2))
                        # sb = ln(sigma*sqrt2); sigma read strided from pl
                        nc.scalar.activation(sb[:, half], plv[:, :, 1], AF.Ln, scale=sqrt2)
                        # a2 = logbid - mu (bf16 out); mu read strided from pl
                        nc.vector.tensor_sub(a2[:, half], sa[:, half], plv[:, :, 0])
                    # b = 1/(sigma*sqrt2) in bf16
                    nc.scalar.activation(bt[:], sb[:], AF.Exp, scale=-1.0)
                    # mid <- 0.25; iteration 0 (probe at the constant 0.5)
                    # finishes it to {0.25, 0.75} via the fused step op
                    nc.gpsimd.memset(mid[:], 0.25)
                    members.append((gi, mid, a2, bt))

                ln_half = float(np.log(0.5))
                scratch = {}
                for k in range(n_iter):
                    w = float(2.0 ** (-(k + 2)))
                    # stage Ln (natural_log_exp set); k=0 probes the constant
                    # d=0.5, so Ln(0.5) folds into a tensor_scalar (4x mode)
                    for gi, mid, a2, bt in members:
                        t1 = p_t1.tile([P, fd], dtb, tag="t1")
                        t2 = p_t2.tile([P, fd], dt, tag="t2")
                        t3 = p_t3.tile([P, fd], dtb, tag="t3")
                        scratch[gi] = (t1, t2, t3)
                        if k > 0:
                            nc.scalar.activation(t1[:], mid[:], AF.Ln)
                    # stage t = (L + a2) * b   (bf16 tensor_tensor -> 2x mode)
                    for gi, mid, a2, bt in members:
                        t1, t2, t3 = scratch[gi]
                        if k == 0:
                            nc.vector.tensor_scalar(
                                t1[:], a2[:], ln_half, None, op0=ALU.add
                            )
                        else:
                            nc.vector.tensor_add(t1[:], t1[:], a2[:])
                        nc.vector.tensor_mul(t1[:], t1[:], bt[:])
                    # stage E = erf'(t) = (2/sqrt(pi))e^{-t^2}  (erf_derivative set)
                    for gi, mid, a2, bt in members:
                        t1, t2, t3 = scratch[gi]
                        nc.scalar.activation(t3[:], t1[:], AF.Derivative_Erf)
                    # stage Erf (sigmoid set — one swap; swap back next iter)
                    for gi, mid, a2, bt in members:
                        t1, t2, t3 = scratch[gi]
                        nc.scalar.activation(t2[:], t1[:], AF.Erf)
                    if k == 0:
                        # mid==0.5 exactly: sign(g) = sign((ef+1) - u), and
                        # mid' = (0.25-0) + (g<=0)*0.5 via the fused step op
                        for gi, mid, a2, bt in members:
                            t1, t2, t3 = scratch[gi]
                            nc.vector.tensor_mul(t3[:], t3[:], bt[:])
                            nc.vector.scalar_tensor_tensor(
                                t2[:], t2[:], 1.0, t3[:],
                                op0=ALU.add, op1=ALU.subtract,
                            )
                            nc.vector._custom_dve(
                                step_op, out=mid[:], in0=t2[:], in1=mid[:],
                                s0=0.0, s1=0.0, imm2=0.5,
                            )
                        continue
                    # stage decide, S-form: g = mid*(u+ef+1) - u.
                    #   V = ef + u      on PE (fp32 ef + bf16 u, exact)
                    #   P = (V+1)*mid   one stt, fp32 internal (the ef+1
                    #                   cancellation stays in fp32) -> bf16
                    #   g = P - u       on PE via negated identity, into the
                    #                   same PSUM tile (WAR handled by tile)
                    for gi, mid, a2, bt in members:
                        t1, t2, t3 = scratch[gi]
                        # u = b * E   (bf16 -> 2x mode)
                        nc.vector.tensor_mul(t3[:], t3[:], bt[:])
                    for gi, mid, a2, bt in members:
                        t1, t2, t3 = scratch[gi]
                        gps = p_g.tile([P, fd], dt, tag="g")
                        scratch[gi] = (t2, t3, gps)
                        for j in range(n_blk):
                            blk = slice(j * 512, (j + 1) * 512)
                            nc.tensor.matmul(
                                gps[:, blk], identf[:], t2[:, blk],
                                start=True, stop=False,
                            )
                            nc.tensor.matmul(
                                gps[:, blk], ident[:], t3[:, blk],
                                start=False, stop=True,
                            )
                    for gi, mid, a2, bt in members:
                        t2, t3, gps = scratch[gi]
                        c1 = p_c1.tile([P, fd], dtb, tag="c1")
                        scratch[gi] = (t3, c1, gps)
                        # P = (V + 1) * mid -> bf16
                        nc.vector.scalar_tensor_tensor(
                            c1[:], gps[:], 1.0, mid[:],
                            op0=ALU.add, op1=ALU.mult,
                        )
                    for gi, mid, a2, bt in members:
                        t3, c1, gps = scratch[gi]
                        for j in range(n_blk):
                            blk = slice(j * 512, (j + 1) * 512)
                            nc.tensor.matmul(
                                gps[:, blk], ident[:], c1[:, blk],
                                start=True, stop=False,
                            )
                            nc.tensor.matmul(
                                gps[:, blk], identn[:], t3[:, blk],
                                start=False, stop=True,
                            )
                    for gi, mid, a2, bt in members:
                        t3, c1, gps = scratch[gi]
                        # mid = (mid - w) + (g <= 0) * 2w    [fused custom op]
                        nc.vector._custom_dve(
                            step_op, out=mid[:], in0=gps[:], in1=mid[:],
                            s0=0.0, s1=w, imm2=2.0 * w,
                        )

                # ---- finish: store ----
                for gi, mid, a2, bt in members:
                    nc.sync.dma_start(out_v[gi], mid[:])

    nc.finalize()
    return nc


_CACHED = {}


def _get_nc(n_per_core, fd, group_size=GROUP):
    key = (n_per_core, fd, group_size)
    if key not in _CACHED:
        _CACHED[key] = _build_nc(n_per_core, fd, group_size)
    return _CACHED[key]


def kernel(params: np.ndarray, bid_prices: np.ndarray) -> np.ndarray:
    from concourse.bass_utils import run_bass_kernel_spmd

    params = np.ascontiguousarray(params, dtype=np.float32)
    bid_prices = np.ascontiguousarray(bid_prices, dtype=np.float32)
    n = bid_prices.shape[0]
    n_per_core = n // N_CORES

    nc = _get_nc(n_per_core, FD)

    in_maps = []
    for i in range(N_CORES):
        sl = slice(i * n_per_core, (i + 1) * n_per_core)
        in_maps.append({"params": params[sl], "bids": bid_prices[sl]})

    res = run_bass_kernel_spmd(nc, in_maps, core_ids=list(range(N_CORES)))
    return np.concatenate([r["out"] for r in res.results], axis=0)


if __name__ == "__main__":
    # smoke test with random data
    rng = np.random.RandomState(0)
    n = N_TOTAL
    params = np.stack(
        [rng.randn(n).astype(np.float32),
         rng.uniform(0.2, 1.5, n).astype(np.float32)], axis=-1
    )
    bids = rng.uniform(0.1, 10.0, n).astype(np.float32)
    out = kernel(params=params, bid_prices=bids)
    print("out", out.shape, out.dtype, out[:8])


# revision 41
# speedup vs baseline: 1.1997x; 1.1997x over previous
"""Trainium2 Bass kernel: per-element maximization of the lognormal-CDF
surplus  s(d) = bid*(1-d)*Phi((ln(d*bid)-mu)/sigma)  over d in (0,1).

Algorithm change vs the GSS reference: the surplus is strictly unimodal in d
(its log-derivative is a decreasing function), so the reference's 20-iteration
golden-section search output is within phi^-20 ~ 6.6e-5 of the root of the
first-order condition.  We binary-search the FOC sign directly:

    sign(ds/dd) = sign( (1-d)*B*E(t) - (1+erf(t))*d ),
    t = (ln d + A2)*B,  A2 = ln(bid)-mu,  B = 1/(sigma*sqrt2),
    E(t) = (2/sqrt(pi))*exp(-t^2)

One probe per iteration (vs two surplus evaluations for GSS) and 0.5x interval
shrink per iteration (vs 0.618x) means 7 probes reach 2^-8 ~ 3.9e-3 accuracy;
measured rel_l2 vs the reference is 5.6e-3 on the full 16.7M-element input
(tolerance 2e-2).  Ties (both FOC terms underflow to 0 when the CDF is fully
saturated) step right, matching the reference's cond=False branch.  Probe 0 is
at the constant d=0.5, so its Ln folds into a tensor_scalar and its decision
needs no mid factors.

Engine mapping per iteration per [128, FD] chunk:
  ACT: Ln(mid) -> bf16, E=Derivative_Erf(t) -> bf16, Erf(t) -> fp32
       (3 table sets; get_activation_tables is patched - contents only,
        order/ids preserved - so the chooser never bounces between the
        per-function home sets)
  VE : t=(L+a2)*b (2 bf16 tensor_tensor at 2x), u=b*E (bf16 2x),
       cmp1=(mid-1)*u -> bf16 (stt, fp32 internal), cmp2=(1+erf)*mid -> bf16
       (stt; the +1 must precede any bf16 rounding - quantizing erf directly
        cancels catastrophically near erf=-1), fused custom-DVE update
       mid' = (mid - w) + (g<=0)*2w reading g from PSUM (1 op)
  PE : g = cmp1 + cmp2 as identity-weight bf16 matmuls accumulated in PSUM
       (exact fp32 sum of the bf16 operands; TensorE is otherwise idle)
mid stays fp32 (needs 2^-9 resolution).  GpSimd is only used for memsets:
its elementwise ops share the DVE SBUF port and inflate Vector op latency.
Chunks are processed in interleaved pairs, stages interleaved across the
pair, so ACT/PE work on one chunk hides under VE work on the other.
"""
import sys

sys.path.insert(0, "/opt/trn_rl_repo")

import numpy as np

N_TOTAL = 16777216
N_CORES = 8
N_PER_CORE = N_TOTAL // N_CORES  # 2097152
P = 128
FD = 1024
GROUP = 4
N_ITER = 7

_CUSTOM_OP = None


def _get_custom_op():
    """Register the fused bisection-update DVE op (idempotent)."""
    global _CUSTOM_OP
    if _CUSTOM_OP is not None:
        return _CUSTOM_OP
    import concourse.dve_ops as dops
    from concourse.dve_spec import Spec, Src0, Src1, C1, C2, Zero, lower
    from concourse.dve_uop import DveOpSpec

    name = "BISECT_STEP_ANT"
    if name in dops._SUB_OPCODE_FOR_NAME:
        _CUSTOM_OP = next(op for op in dops.OPS if op.name == name)
        return _CUSTOM_OP

    body = (Src1 - C1) + (Src0 <= Zero) * C2

    def _ref(in0, in1, s0, s1, imm2):
        return ((in1.astype(np.float32) - s1)
                + (in0 <= 0.0).astype(np.float32) * imm2).astype(np.float32)

    spec = Spec(body=body, reference=_ref)
    row = max(dops._SUB_OPCODE_FOR_NAME.values()) + 1
    assert row < 0x20
    shas = {}
    for ver in ("v3", "v4"):
        uops = lower(spec, ver=ver)
        shas[ver] = DveOpSpec(
            name=name, opcode=row, uops=uops, rd1_en=True
        ).sha(ver)
    op = dops.DveOp(name, spec, subdim=False, uops_sha=shas)
    dops.OPS.append(op)
    dops.CUSTOM_DVE_SPECS[name] = spec
    dops._SUB_OPCODE_FOR_NAME[name] = row
    _CUSTOM_OP = op
    return op


def _patch_act_table_order():
    """Make the act-table chooser use one set for Ln+Square+Exp.

    The dict insertion order of get_activation_tables IS the
    act_func_set_id walrus uses, so the dict may not be reordered.
    Instead, prune Ln/Square/Exp from every set except
    natural_log_exp_and_others (and Square also stays in
    sigmoid_and_others, Erf's set) so the chooser has no other choice.
    The physical tables are unchanged — we only narrow the chooser's
    view — so the pruned functions still work from the kept sets."""
    import concourse.bacc as bacc
    import concourse.hw_specs as hw
    import concourse.mybir as mybir

    if getattr(bacc, "_ant_table_patched", False):
        return
    orig = hw.get_activation_tables
    AF = mybir.ActivationFunctionType

    def pruned(arch):
        t = {k: set(v) for k, v in orig(arch).items()}
        keep_ln_exp = "natural_log_exp_and_others"
        for name, fns in t.items():
            if name != keep_ln_exp:
                fns.discard(AF.Ln)
                fns.discard(AF.Exp)
                fns.discard(AF.Square)
        return t

    bacc.get_activation_tables = pruned
    bacc._ant_table_patched = True


def _build_nc(n_per_core, fd, group_size=GROUP, n_iter=N_ITER):
    import concourse.bass as bass
    import concourse.bacc as bacc
    import concourse.mybir as mybir
    import concourse.tile as tile

    _patch_act_table_order()

    from concourse import masks

    AF = mybir.ActivationFunctionType
    ALU = mybir.AluOpType
    dt = mybir.dt.float32
    dtb = mybir.dt.bfloat16

    step_op = _get_custom_op()

    n_chunks = n_per_core // (P * fd)
    assert n_chunks * P * fd == n_per_core

    nc = bacc.Bacc(None, target_bir_lowering=False)

    exp_bias = float(np.log(2.0 / np.sqrt(np.pi)))

    # non-Copy activation float biases must exist as const APs
    def register_const(value: float):
        if (dt, value) in nc.const_aps.aps:
            return
        t = nc.alloc_sbuf_tensor(f"const-f32-c{len(nc.const_aps.aps)}", [128, 1], dt)
        nc.gpsimd.memset(t.ap(), value)
        nc.const_aps.aps[(dt, value)] = t.ap()

    register_const(exp_bias)
    nc.all_engine_barrier()

    params = nc.declare_dram_parameter("params", [n_per_core, 2], dt, isOutput=False)
    bids = nc.declare_dram_parameter("bids", [n_per_core], dt, isOutput=False)
    out = nc.declare_dram_parameter("out", [n_per_core], dt, isOutput=True)

    # contiguous [G, 128, 2*fd] view of interleaved (mu, sigma) pairs
    params_v = params.rearrange("(g p f) c -> g p (f c)", p=P, f=fd)
    bids_v = bids.rearrange("(g p f) -> g p f", p=P, f=fd)
    out_v = out.rearrange("(g p f) -> g p f", p=P, f=fd)

    sqrt2 = float(np.sqrt(2.0))

    n_blk = fd // 512  # PSUM-bank-sized matmul blocks

    with tile.TileContext(nc) as tc:
        with (
            tc.tile_pool(name="ident", bufs=1) as p_id,
            tc.tile_pool(name="st_mid", bufs=2 * group_size) as p_mid,
            tc.tile_pool(name="st_a2", bufs=2 * group_size) as p_a2,
            tc.tile_pool(name="st_b", bufs=2 * group_size) as p_b,
            tc.tile_pool(name="t1", bufs=group_size + 1) as p_t1,
            tc.tile_pool(name="t2", bufs=group_size) as p_t2,
            tc.tile_pool(name="t3", bufs=group_size + 1) as p_t3,
            tc.tile_pool(name="c1", bufs=group_size + 1) as p_c1,
            tc.tile_pool(name="c2", bufs=group_size + 1) as p_c2,
            tc.tile_pool(name="gp", bufs=group_size, space="PSUM") as p_g,
            tc.tile_pool(name="sa", bufs=group_size) as p_sa,
            tc.tile_pool(name="sb", bufs=group_size) as p_sb,
            tc.tile_pool(name="pload", bufs=group_size - 1) as p_pl,
        ):
            ident = p_id.tile([P, P], dtb, tag="ident")
            masks.make_identity(nc, ident[:])
            for g0 in range(0, n_chunks, group_size):
                members = []
                # ---- setup each chunk of the group ----
                for gi in range(g0, min(g0 + group_size, n_chunks)):
                    mid = p_mid.tile([P, fd], dt, tag="mid")
                    a2 = p_a2.tile([P, fd], dtb, tag="a2")
                    bt = p_b.tile([P, fd], dtb, tag="b")
                    sa = p_sa.tile([P, fd], dt, tag="sa")
                    sb = p_sb.tile([P, fd], dt, tag="sb")
                    # load bid -> sa (scratch), then logbid in place
                    nc.sync.dma_start(sa[:], bids_v[gi])
                    nc.scalar.activation(sa[:], sa[:], AF.Ln)
                    # interleaved params arrive in two [P, fd] staging halves
                    for h in range(2):
                        pl = p_pl.tile([P, fd], dt, tag="pl")
                        nc.sync.dma_start(pl[:], params_v[gi, :, h * fd:(h + 1) * fd])
                        plv = pl.rearrange("p (f c) -> p f c", c=2)
                        half = slice(h * (fd // 2), (h + 1) * (fd // 2))
                        # sb = ln(sigma*sqrt2); sigma read strided from pl
                        nc.scalar.activation(sb[:, half], plv[:, :, 1], AF.Ln, scale=sqrt2)
                        # sa = logbid - mu; mu read strided from pl
                        nc.vector.tensor_sub(sa[:, half], sa[:, half], plv[:, :, 0])
                    # bf16 working copies: a2 = ln(bid)-mu, b = 1/(sigma*sqrt2)
                    nc.scalar.activation(a2[:], sa[:], AF.Copy)
                    nc.scalar.activation(bt[:], sb[:], AF.Exp, scale=-1.0)
                    # mid <- 0.25; iteration 0 (probe at the constant 0.5)
                    # finishes it to {0.25, 0.75} via the fused step op
                    nc.gpsimd.memset(mid[:], 0.25)
                    members.append((gi, mid, a2, bt))

                ln_half = float(np.log(0.5))
                scratch = {}
                for k in range(n_iter):
                    w = float(2.0 ** (-(k + 2)))
                    # stage Ln (natural_log_exp set); k=0 probes the constant
                    # d=0.5, so Ln(0.5) folds into a tensor_scalar (4x mode)
                    for gi, mid, a2, bt in members:
                        t1 = p_t1.tile([P, fd], dtb, tag="t1")
                        t2 = p_t2.tile([P, fd], dt, tag="t2")
                        t3 = p_t3.tile([P, fd], dtb, tag="t3")
                        scratch[gi] = (t1, t2, t3)
                        if k > 0:
                            nc.scalar.activation(t1[:], mid[:], AF.Ln)
                    # stage t = (L + a2) * b   (bf16 tensor_tensor -> 2x mode)
                    for gi, mid, a2, bt in members:
                        t1, t2, t3 = scratch[gi]
                        if k == 0:
                            nc.vector.tensor_scalar(
                                t1[:], a2[:], ln_half, None, op0=ALU.add
                            )
                        else:
                            nc.vector.tensor_add(t1[:], t1[:], a2[:])
                        nc.vector.tensor_mul(t1[:], t1[:], bt[:])
                    # stage E = erf'(t) = (2/sqrt(pi))e^{-t^2}  (erf_derivative set)
                    for gi, mid, a2, bt in members:
                        t1, t2, t3 = scratch[gi]
                        nc.scalar.activation(t3[:], t1[:], AF.Derivative_Erf)
                    # stage Erf (sigmoid set — one swap; swap back next iter)
                    for gi, mid, a2, bt in members:
                        t1, t2, t3 = scratch[gi]
                        nc.scalar.activation(t2[:], t1[:], AF.Erf)
                    if k == 0:
                        # mid==0.5 exactly: sign(g) = sign((ef+1) - u), and
                        # mid' = (0.25-0) + (g<=0)*0.5 via the fused step op
                        for gi, mid, a2, bt in members:
                            t1, t2, t3 = scratch[gi]
                            nc.vector.tensor_mul(t3[:], t3[:], bt[:])
                            nc.vector.scalar_tensor_tensor(
                                t2[:], t2[:], 1.0, t3[:],
                                op0=ALU.add, op1=ALU.subtract,
                            )
                            nc.vector._custom_dve(
                                step_op, out=mid[:], in0=t2[:], in1=mid[:],
                                s0=0.0, s1=0.0, imm2=0.5,
                            )
                        continue
                    # stage decide: stages interleaved across members so the
                    # Vector engine fills one member's Erf-wait with the
                    # other member's earlier work
                    for gi, mid, a2, bt in members:
                        t1, t2, t3 = scratch[gi]
                        # u = b * E   (bf16 -> 2x mode)
                        nc.vector.tensor_mul(t3[:], t3[:], bt[:])
                    for gi, mid, a2, bt in members:
                        t1, t2, t3 = scratch[gi]
                        c1 = p_c1.tile([P, fd], dtb, tag="c1")
                        # cmp1 = (mid - 1) * u  -> bf16   ( = -(1-d) b E )
                        nc.vector.scalar_tensor_tensor(
                            c1[:], mid[:], 1.0, t3[:],
                            op0=ALU.subtract, op1=ALU.mult,
                        )
                        scratch[gi] = (t1, t2, t3, c1)
                    for gi, mid, a2, bt in members:
                        t1, t2, t3, c1 = scratch[gi]
                        c2 = p_c2.tile([P, fd], dtb, tag="c2")
                        gps = p_g.tile([P, fd], dt, tag="g")
                        scratch[gi] = (c1, c2, gps)
                        # cmp2 = (erf + 1) * mid -> bf16  ( = (1+erf) d )
                        nc.vector.scalar_tensor_tensor(
                            c2[:], t2[:], 1.0, mid[:],
                            op0=ALU.add, op1=ALU.mult,
                        )
                    # g = cmp1 + cmp2 via identity matmuls (TensorE, exact
                    # fp32 accumulate of the bf16 inputs); g <= 0 -> right
                    for gi, mid, a2, bt in members:
                        c1, c2, gps = scratch[gi]
                        for j in range(n_blk):
                            blk = slice(j * 512, (j + 1) * 512)
                            nc.tensor.matmul(
                                gps[:, blk], ident[:], c1[:, blk],
                                start=True, stop=False,
                            )
                            nc.tensor.matmul(
                                gps[:, blk], ident[:], c2[:, blk],
                                start=False, stop=True,
                            )
                    for gi, mid, a2, bt in members:
                        c1, c2, gps = scratch[gi]
                        # mid = (mid - w) + (g <= 0) * 2w    [fused custom op]
                        nc.vector._custom_dve(
                            step_op, out=mid[:], in0=gps[:], in1=mid[:],
                            s0=0.0, s1=w, imm2=2.0 * w,
                        )

                # ---- finish: store ----
                for gi, mid, a2, bt in members:
                    nc.sync.dma_start(out_v[gi], mid[:])

    nc.finalize()
    return nc


_CACHED = {}


def _get_nc(n_per_core, fd, group_size=GROUP):
    key = (n_per_core, fd, group_size)
    if key not in _CACHED:
        _CACHED[key] = _build_nc(n_per_core, fd, group_size)
    return _CACHED[key]


def kernel(params: np.ndarray, bid_prices: np.ndarray) -> np.ndarray:
    from concourse.bass_utils import run_bass_kernel_spmd

    params = np.ascontiguousarray(params, dtype=np.float32)
    bid_prices = np.ascontiguousarray(bid_prices, dtype=np.float32)
    n = bid_prices.shape[0]
    n_per_core = n // N_CORES

    nc = _get_nc(n_per_core, FD)

    in_maps = []
    for i in range(N_CORES):
        sl = slice(i * n_per_core, (i + 1) * n_per_core)
        in_maps.append({"params": params[sl], "bids": bid_prices[sl]})

    res = run_bass_kernel_spmd(nc, in_maps, core_ids=list(range(N_CORES)))
    return np.concatenate([r["out"] for r in res.results], axis=0)


if __name__ == "__main__":
    # smoke test with random data
    rng = np.random.RandomState(0)
    n = N_TOTAL
    params = np.stack(
        [rng.randn(n).astype(np.float32),
         rng.uniform(0.2, 1.5, n).astype(np.float32)], axis=-1
    )
    bids = rng.uniform(0.1, 10.0, n).astype(np.float32)
    out = kernel(params=params, bid_prices=bids)
    print("out", out.shape, out.dtype, out[:8])
